# revision 6
# baseline (speedup 1.0000x reference)
"""Trainium2 Bass kernel for nn_Jitter: block-wise bilinear jitter (grid_sample).

Math (per sample s, 16x16 block (by,bx), PROB=1.0, align_corners=True):
  dx = 511*rx - 255.5, dy = 511*ry - 255.5   (rx,ry = random_flow_lr in [0,1))
  out[c, 16by+ii, 16bx+jj] = bilinear(x[c], y=16by+ii+dy, x=16bx+jj+dx), zero pad.
Since floor(j+dx) = j+floor(dx), each block needs a 17x17 source window at
integer offset (floor(dy), floor(dx)) and constant fractional weights (wy, wx).

v2 design (pure data parallel, 4 samples/core on 8 cores):
  - Host stages x as fp16 panels with the 3 channels interleaved per row:
    panel t covers cols [8t-17, 8t+9) (stride 8, 26 cols/section, 78/row),
    rows -17..529, zero-padded on every edge. With coordinates clamped to
    c0 in [-17,512], r0 in [-17,513], every out-of-bounds tap reads a real
    zero -- no validity masks anywhere.
  - One indirect DMA (idx tile [P,8]) gathers 8 bx-blocks x 128 windows of
    17x78 fp16 (all 3 channels) -> 4 gather instructions/core instead of 96
    (SWDGE has ~2us fixed software overhead per instruction).
  - Blend per bx with per-partition scalar weights:
      ScalarE: tmp1 = win[1:17]*wy, tmp2 = win[0:16]*(1-wy)  (activation.Copy)
      Vector:  s = tmp1+tmp2 (2x), s_shift = copy(s[...,1:17]) (2x_2p),
               a = s[...,0:16]*(1-wx) (4x),
               out_f32 = s_shift*wx + a (scalar_tensor_tensor).
  - Output accumulated in [P,16,3,128] f32 tiles, 12 HWDGE writes of 512B runs.
"""

import numpy as np

import concourse.bacc as bacc
import concourse.bass as bass
import concourse.mybir as mybir
import concourse.tile as tile
from concourse.bass import IndirectOffsetOnAxis
from concourse.bass_utils import run_bass_kernel_spmd

F32 = mybir.dt.float32
F16 = mybir.dt.float16
I32 = mybir.dt.int32

B, C, H, W = 32, 3, 512, 512
NCORES = 8
S = B // NCORES            # 4 samples per core
NBY, NBX = H // 16, W // 16
P = S * NBY                # 128 partitions = (s, by)
STR = 8                    # panel column stride
SEC = STR + 18             # 26 cols per channel section
ROWW = C * SEC             # 78 elems per panel row
NPAN = (512 + 17) // STR + 1   # 67 panels
PR = 17 + H + 18           # 547 rows per panel
PANEL = PR * ROWW          # 42666
SPLANE = NPAN * PANEL      # 2858622 per sample
FPAD = 2048
XSN = FPAD + S * SPLANE + FPAD
ELEM = 17 * ROWW           # 1326 fp16 gathered per window
KC = 8                     # bx per gather instruction
NG = NBX // KC             # 4 gather groups

_CACHE = {}


def _build_consts():
    ct = np.zeros((P, 35), dtype=np.float32)
    ct[:, 0:32] = (16.0 * np.arange(32) - 256.0)[None, :]        # 16bx-256
    p = np.arange(P)
    ct[:, 32] = 16.0 * (p % NBY) - 256.0                         # 16by-256
    ct[:, 33] = float(FPAD) + (p // NBY) * float(SPLANE)         # sample base
    return ct


def _stage_panels(xs_core):
    """xs_core: [S,3,512,512] -> staged fp16 panel buffer [XSN]."""
    x16 = np.ascontiguousarray(xs_core).astype(np.float16)
    xpad = np.zeros((S, C, PR, 17 + W + SEC), dtype=np.float16)
    xpad[:, :, 17:17 + H, 17:17 + W] = x16
    out = np.zeros(XSN, dtype=np.float16)
    body = out[FPAD:FPAD + S * SPLANE].reshape(S, NPAN, PR, C, SEC)
    for t in range(NPAN):
        body[:, t] = xpad[:, :, :, STR * t:STR * t + SEC].transpose(0, 2, 1, 3)
    return out


def _build_nc():
    nc = bacc.Bacc("TRN2", target_bir_lowering=False, debug=False,
                   num_devices=NCORES)

    xs = nc.dram_tensor("xs", [XSN, 1], F16, kind="ExternalInput")
    flow = nc.dram_tensor("flow", [S, 2, NBY, NBX], F32, kind="ExternalInput")
    cst = nc.dram_tensor("cst", [P, 35], F32, kind="ExternalInput")
    y = nc.dram_tensor("y", [S, C, H, W], F32, kind="ExternalOutput")

    with tile.TileContext(nc) as tc:
        with (
            tc.tile_pool(name="prep", bufs=1) as pp,
            tc.tile_pool(name="win", bufs=2) as wp,
            tc.tile_pool(name="blend", bufs=2) as lp,
            tc.tile_pool(name="out", bufs=2) as op,
        ):
            v = nc.vector
            A = mybir.AluOpType

            ct = pp.tile([P, 35], F32, tag="ct")
            nc.sync.dma_start(ct[:], cst[:])
            fxt = pp.tile([P, NBX], F32, tag="fxt")
            fyt = pp.tile([P, NBX], F32, tag="fyt")
            fl = flow[:]
            nc.sync.dma_start(fxt[:], fl[:, 0])
            nc.sync.dma_start(fyt[:], fl[:, 1])

            # v* = 511*r + 0.5 (>= 0, so trunc-based floor works)
            vx = pp.tile([P, NBX], F32, tag="vx")
            vy = pp.tile([P, NBX], F32, tag="vy")
            v.tensor_scalar(vx[:], fxt[:], 511.0, 0.5, A.mult, A.add)
            v.tensor_scalar(vy[:], fyt[:], 511.0, 0.5, A.mult, A.add)

            def floor_pos(src, tag):
                """floor(src) for src >= 0, robust to any cast rounding."""
                ti = pp.tile([P, NBX], I32, tag=tag + "i")
                tf = pp.tile([P, NBX], F32, tag=tag + "f")
                ad = pp.tile([P, NBX], F32, tag=tag + "a")
                out = pp.tile([P, NBX], F32, tag=tag + "o")
                v.tensor_copy(ti[:], src[:])
                v.tensor_copy(tf[:], ti[:])
                v.tensor_tensor(ad[:], tf[:], src[:], A.is_gt)
                v.tensor_tensor(out[:], tf[:], ad[:], A.subtract)
                return out

            flx = floor_pos(vx, "flx")   # floor(dx)+256 in [0, 511]
            fly = floor_pos(vy, "fly")
            # weights (per-partition scalars per bx)
            wxb = pp.tile([P, NBX], F32, tag="wxb")
            wyb = pp.tile([P, NBX], F32, tag="wyb")
            wxa = pp.tile([P, NBX], F32, tag="wxa")
            wya = pp.tile([P, NBX], F32, tag="wya")
            v.tensor_tensor(wxb[:], vx[:], flx[:], A.subtract)
            v.tensor_tensor(wyb[:], vy[:], fly[:], A.subtract)
            v.tensor_scalar(wxa[:], wxb[:], -1.0, 1.0, A.mult, A.add)
            v.tensor_scalar(wya[:], wyb[:], -1.0, 1.0, A.mult, A.add)

            # c0 = flx + 16bx - 256, clamped to [-17, 512]; u = c0+17
            c0 = pp.tile([P, NBX], F32, tag="c0")
            v.tensor_tensor(c0[:], flx[:], ct[:, 0:32], A.add)
            u = pp.tile([P, NBX], F32, tag="u")
            v.tensor_scalar(u[:], c0[:], 17.0, 0.0, A.add, A.max)
            v.tensor_scalar(u[:], u[:], 529.0, None, A.min)
            # t = floor(u/8), o = u - 8t
            t8 = pp.tile([P, NBX], F32, tag="t8")
            v.tensor_scalar(t8[:], u[:], 0.125, None, A.mult)
            tfl = floor_pos(t8, "tfl")
            o_t = pp.tile([P, NBX], F32, tag="o_t")
            v.tensor_scalar(o_t[:], tfl[:], -8.0, None, A.mult)
            v.tensor_tensor(o_t[:], o_t[:], u[:], A.add)

            # r0 = fly + 16by - 256 clamped to [-17, 513]; rr = r0+17
            rr = pp.tile([P, NBX], F32, tag="rr")
            v.tensor_scalar(rr[:], fly[:], ct[:, 32:33], None, A.add)
            v.tensor_scalar(rr[:], rr[:], 17.0, 0.0, A.add, A.max)
            v.tensor_scalar(rr[:], rr[:], 530.0, None, A.min)

            # idx = sbase + t*PANEL + rr*ROWW + o   (all exact in f32, < 2^24)
            idxf = pp.tile([P, NBX], F32, tag="idxf")
            v.tensor_scalar(idxf[:], tfl[:], float(PANEL), ct[:, 33:34],
                            A.mult, A.add)
            t2 = pp.tile([P, NBX], F32, tag="t2")
            v.tensor_scalar(t2[:], rr[:], float(ROWW), None, A.mult)
            v.tensor_tensor(idxf[:], idxf[:], t2[:], A.add)
            v.tensor_tensor(idxf[:], idxf[:], o_t[:], A.add)
            idxi = pp.tile([P, NBX], I32, tag="idxi")
            v.tensor_copy(idxi[:], idxf[:])

            Copy = mybir.ActivationFunctionType.Copy
            yv = y[:]
            for g in range(NG):
                win = wp.tile([P, KC, ELEM], F16, tag="win")
                for k in range(KC):
                    bx = g * KC + k
                    nc.gpsimd.indirect_dma_start(
                        out=win[:, k, :], out_offset=None,
                        in_=xs[:],
                        in_offset=IndirectOffsetOnAxis(
                            ap=idxi[:][:, bx:bx + 1], axis=0),
                    )
                outt = op.tile([P, C, 16, 128], F32, tag="outt")
                for k in range(KC):
                    bx = g * KC + k
                    w4 = win[:][:, k].rearrange("p (ii c w) -> p c ii w",
                                                c=C, w=SEC)
                    tmp1 = lp.tile([P, C, 16, 18], F16, tag="tmp1")
                    tmp2 = lp.tile([P, C, 16, 18], F16, tag="tmp2")
                    nc.scalar.activation(tmp1[:], w4[:, :, 1:17, 0:18], Copy,
                                         scale=wyb[:, bx:bx + 1])
                    if k % 2 == 0:
                        nc.scalar.activation(tmp2[:], w4[:, :, 0:16, 0:18],
                                             Copy, scale=wya[:, bx:bx + 1])
                    else:
                        v.tensor_scalar(tmp2[:], w4[:, :, 0:16, 0:18],
                                        wya[:, bx:bx + 1], None, A.mult)
                    s = lp.tile([P, C, 16, 18], F16, tag="s")
                    v.tensor_tensor(s[:], tmp1[:], tmp2[:], A.add)
                    av = lp.tile([P, C, 16, 16], F16, tag="av")
                    v.tensor_scalar(av[:], s[:, :, :, 0:16],
                                    wxa[:, bx:bx + 1], None, A.mult)
                    v.scalar_tensor_tensor(
                        outt[:, :, :, k * 16:(k + 1) * 16],
                        s[:, :, :, 1:17], wxb[:, bx:bx + 1], av[:],
                        A.mult, A.add)
                for c in range(C):
                    yc = yv[:, c].rearrange("s (by ii) w -> s by ii w", ii=16)
                    nc.sync.dma_start(
                        out=yc[:, :, :, g * KC * 16:(g + 1) * KC * 16],
                        in_=outt[:, c, :, :])

    nc.compile()
    return nc


def get_nc():
    if "nc" not in _CACHE:
        _CACHE["nc"] = _build_nc()
    return _CACHE["nc"]


def make_in_maps(x, random_flow_lr):
    x = np.ascontiguousarray(x, dtype=np.float32)
    rfl = np.ascontiguousarray(random_flow_lr, dtype=np.float32)
    cst = _build_consts()
    in_maps = []
    for k in range(NCORES):
        xsb = _stage_panels(x[k * S:(k + 1) * S]).reshape(XSN, 1)
        in_maps.append({"xs": xsb, "flow": rfl[k * S:(k + 1) * S], "cst": cst})
    return in_maps


def kernel(x, random_flow_lr):
    nc = get_nc()
    in_maps = make_in_maps(x, random_flow_lr)
    res = run_bass_kernel_spmd(nc, in_maps, core_ids=list(range(NCORES)))
    return np.concatenate([r["y"] for r in res.results], axis=0)


# revision 7
# speedup vs baseline: 1.0156x; 1.0156x over previous
"""Trainium2 Bass kernel for nn_Jitter: block-wise bilinear jitter (grid_sample).

Math (per sample s, 16x16 block (by,bx), PROB=1.0, align_corners=True):
  dx = 511*rx - 255.5, dy = 511*ry - 255.5   (rx,ry = random_flow_lr in [0,1))
  out[c, 16by+ii, 16bx+jj] = bilinear(x[c], y=16by+ii+dy, x=16bx+jj+dx), zero pad.
Since floor(j+dx) = j+floor(dx), each block needs a 17x17 source window at
integer offset (floor(dy), floor(dx)) and constant fractional weights (wy, wx).

v2 design (pure data parallel, 4 samples/core on 8 cores):
  - Host stages x as fp16 panels with the 3 channels interleaved per row:
    panel t covers cols [8t-17, 8t+9) (stride 8, 26 cols/section, 78/row),
    rows -17..529, zero-padded on every edge. With coordinates clamped to
    c0 in [-17,512], r0 in [-17,513], every out-of-bounds tap reads a real
    zero -- no validity masks anywhere.
  - One indirect DMA (idx tile [P,8]) gathers 8 bx-blocks x 128 windows of
    17x78 fp16 (all 3 channels) -> 4 gather instructions/core instead of 96
    (SWDGE has ~2us fixed software overhead per instruction).
  - Blend per bx with per-partition scalar weights:
      ScalarE: tmp1 = win[1:17]*wy, tmp2 = win[0:16]*(1-wy)  (activation.Copy)
      Vector:  s = tmp1+tmp2 (2x), s_shift = copy(s[...,1:17]) (2x_2p),
               a = s[...,0:16]*(1-wx) (4x),
               out_f32 = s_shift*wx + a (scalar_tensor_tensor).
  - Output accumulated in [P,16,3,128] f32 tiles, 12 HWDGE writes of 512B runs.
"""

import numpy as np

import concourse.bacc as bacc
import concourse.bass as bass
import concourse.mybir as mybir
import concourse.tile as tile
from concourse.bass import IndirectOffsetOnAxis
from concourse.bass_utils import run_bass_kernel_spmd

F32 = mybir.dt.float32
F16 = mybir.dt.float16
I32 = mybir.dt.int32

B, C, H, W = 32, 3, 512, 512
NCORES = 8
S = B // NCORES            # 4 samples per core
NBY, NBX = H // 16, W // 16
P = S * NBY                # 128 partitions = (s, by)
STR = 8                    # panel column stride
SEC = STR + 18             # 26 cols per channel section
ROWW = C * SEC             # 78 elems per panel row
NPAN = (512 + 17) // STR + 1   # 67 panels
PR = 17 + H + 18           # 547 rows per panel
PANEL = PR * ROWW          # 42666
SPLANE = NPAN * PANEL      # 2858622 per sample
FPAD = 2048
XSN = FPAD + S * SPLANE + FPAD
ELEM = 17 * ROWW           # 1326 fp16 gathered per window
KC = 8                     # bx per gather instruction
NG = NBX // KC             # 4 gather groups

_CACHE = {}


def _build_consts():
    ct = np.zeros((P, 35), dtype=np.float32)
    ct[:, 0:32] = (16.0 * np.arange(32) - 256.0)[None, :]        # 16bx-256
    p = np.arange(P)
    ct[:, 32] = 16.0 * (p % NBY) - 256.0                         # 16by-256
    ct[:, 33] = float(FPAD) + (p // NBY) * float(SPLANE)         # sample base
    return ct


def _stage_panels(xs_core):
    """xs_core: [S,3,512,512] -> staged fp16 panel buffer [XSN]."""
    x16 = np.ascontiguousarray(xs_core).astype(np.float16)
    xpad = np.zeros((S, C, PR, 17 + W + SEC), dtype=np.float16)
    xpad[:, :, 17:17 + H, 17:17 + W] = x16
    out = np.zeros(XSN, dtype=np.float16)
    body = out[FPAD:FPAD + S * SPLANE].reshape(S, NPAN, PR, C, SEC)
    for t in range(NPAN):
        body[:, t] = xpad[:, :, :, STR * t:STR * t + SEC].transpose(0, 2, 1, 3)
    return out


def _build_nc():
    nc = bacc.Bacc("TRN2", target_bir_lowering=False, debug=False,
                   num_devices=NCORES)

    xs = nc.dram_tensor("xs", [XSN, 1], F16, kind="ExternalInput")
    flow = nc.dram_tensor("flow", [S, 2, NBY, NBX], F32, kind="ExternalInput")
    cst = nc.dram_tensor("cst", [P, 35], F32, kind="ExternalInput")
    y = nc.dram_tensor("y", [S, C, H, W], F32, kind="ExternalOutput")

    with tile.TileContext(nc) as tc:
        with (
            tc.tile_pool(name="prep", bufs=1) as pp,
            tc.tile_pool(name="win", bufs=3) as wp,
            tc.tile_pool(name="blend", bufs=3) as lp,
            tc.tile_pool(name="out", bufs=3) as op,
        ):
            v = nc.vector
            A = mybir.AluOpType

            ct = pp.tile([P, 35], F32, tag="ct")
            nc.sync.dma_start(ct[:], cst[:])
            fxt = pp.tile([P, NBX], F32, tag="fxt")
            fyt = pp.tile([P, NBX], F32, tag="fyt")
            fl = flow[:]
            nc.sync.dma_start(fxt[:], fl[:, 0])
            nc.sync.dma_start(fyt[:], fl[:, 1])

            # v* = 511*r + 0.5 (>= 0, so trunc-based floor works)
            vx = pp.tile([P, NBX], F32, tag="vx")
            vy = pp.tile([P, NBX], F32, tag="vy")
            v.tensor_scalar(vx[:], fxt[:], 511.0, 0.5, A.mult, A.add)
            v.tensor_scalar(vy[:], fyt[:], 511.0, 0.5, A.mult, A.add)

            def floor_pos(src, tag):
                """floor(src) for src >= 0, robust to any cast rounding."""
                ti = pp.tile([P, NBX], I32, tag=tag + "i")
                tf = pp.tile([P, NBX], F32, tag=tag + "f")
                ad = pp.tile([P, NBX], F32, tag=tag + "a")
                out = pp.tile([P, NBX], F32, tag=tag + "o")
                v.tensor_copy(ti[:], src[:])
                v.tensor_copy(tf[:], ti[:])
                v.tensor_tensor(ad[:], tf[:], src[:], A.is_gt)
                v.tensor_tensor(out[:], tf[:], ad[:], A.subtract)
                return out

            flx = floor_pos(vx, "flx")   # floor(dx)+256 in [0, 511]
            fly = floor_pos(vy, "fly")
            # weights (per-partition scalars per bx)
            wxb = pp.tile([P, NBX], F32, tag="wxb")
            wyb = pp.tile([P, NBX], F32, tag="wyb")
            wxa = pp.tile([P, NBX], F32, tag="wxa")
            wya = pp.tile([P, NBX], F32, tag="wya")
            v.tensor_tensor(wxb[:], vx[:], flx[:], A.subtract)
            v.tensor_tensor(wyb[:], vy[:], fly[:], A.subtract)
            v.tensor_scalar(wxa[:], wxb[:], -1.0, 1.0, A.mult, A.add)
            v.tensor_scalar(wya[:], wyb[:], -1.0, 1.0, A.mult, A.add)

            # c0 = flx + 16bx - 256, clamped to [-17, 512]; u = c0+17
            c0 = pp.tile([P, NBX], F32, tag="c0")
            v.tensor_tensor(c0[:], flx[:], ct[:, 0:32], A.add)
            u = pp.tile([P, NBX], F32, tag="u")
            v.tensor_scalar(u[:], c0[:], 17.0, 0.0, A.add, A.max)
            v.tensor_scalar(u[:], u[:], 529.0, None, A.min)
            # t = floor(u/8), o = u - 8t
            t8 = pp.tile([P, NBX], F32, tag="t8")
            v.tensor_scalar(t8[:], u[:], 0.125, None, A.mult)
            tfl = floor_pos(t8, "tfl")
            o_t = pp.tile([P, NBX], F32, tag="o_t")
            v.tensor_scalar(o_t[:], tfl[:], -8.0, None, A.mult)
            v.tensor_tensor(o_t[:], o_t[:], u[:], A.add)

            # r0 = fly + 16by - 256 clamped to [-17, 513]; rr = r0+17
            rr = pp.tile([P, NBX], F32, tag="rr")
            v.tensor_scalar(rr[:], fly[:], ct[:, 32:33], None, A.add)
            v.tensor_scalar(rr[:], rr[:], 17.0, 0.0, A.add, A.max)
            v.tensor_scalar(rr[:], rr[:], 530.0, None, A.min)

            # idx = sbase + t*PANEL + rr*ROWW + o   (all exact in f32, < 2^24)
            idxf = pp.tile([P, NBX], F32, tag="idxf")
            v.tensor_scalar(idxf[:], tfl[:], float(PANEL), ct[:, 33:34],
                            A.mult, A.add)
            t2 = pp.tile([P, NBX], F32, tag="t2")
            v.tensor_scalar(t2[:], rr[:], float(ROWW), None, A.mult)
            v.tensor_tensor(idxf[:], idxf[:], t2[:], A.add)
            v.tensor_tensor(idxf[:], idxf[:], o_t[:], A.add)
            idxi = pp.tile([P, NBX], I32, tag="idxi")
            v.tensor_copy(idxi[:], idxf[:])

            Copy = mybir.ActivationFunctionType.Copy
            yv = y[:]
            for g in range(NG):
                win = wp.tile([P, KC, ELEM], F16, tag="win")
                for k in range(KC):
                    bx = g * KC + k
                    nc.gpsimd.indirect_dma_start(
                        out=win[:, k, :], out_offset=None,
                        in_=xs[:],
                        in_offset=IndirectOffsetOnAxis(
                            ap=idxi[:][:, bx:bx + 1], axis=0),
                    )
                outt = op.tile([P, C, 16, 128], F32, tag="outt")
                for k in range(KC):
                    bx = g * KC + k
                    w4 = win[:][:, k].rearrange("p (ii c w) -> p c ii w",
                                                c=C, w=SEC)
                    tmp1 = lp.tile([P, C, 16, 18], F16, tag="tmp1")
                    tmp2 = lp.tile([P, C, 16, 18], F16, tag="tmp2")
                    nc.scalar.activation(tmp1[:], w4[:, :, 1:17, 0:18], Copy,
                                         scale=wyb[:, bx:bx + 1])
                    if k % 2 == 0:
                        nc.scalar.activation(tmp2[:], w4[:, :, 0:16, 0:18],
                                             Copy, scale=wya[:, bx:bx + 1])
                    else:
                        v.tensor_scalar(tmp2[:], w4[:, :, 0:16, 0:18],
                                        wya[:, bx:bx + 1], None, A.mult)
                    s = lp.tile([P, C, 16, 18], F16, tag="s")
                    v.tensor_tensor(s[:], tmp1[:], tmp2[:], A.add)
                    av = lp.tile([P, C, 16, 16], F16, tag="av")
                    v.tensor_scalar(av[:], s[:, :, :, 0:16],
                                    wxa[:, bx:bx + 1], None, A.mult)
                    v.scalar_tensor_tensor(
                        outt[:, :, :, k * 16:(k + 1) * 16],
                        s[:, :, :, 1:17], wxb[:, bx:bx + 1], av[:],
                        A.mult, A.add)
                for c in range(C):
                    yc = yv[:, c].rearrange("s (by ii) w -> s by ii w", ii=16)
                    nc.sync.dma_start(
                        out=yc[:, :, :, g * KC * 16:(g + 1) * KC * 16],
                        in_=outt[:, c, :, :])

    nc.compile()
    return nc


def get_nc():
    if "nc" not in _CACHE:
        _CACHE["nc"] = _build_nc()
    return _CACHE["nc"]


def make_in_maps(x, random_flow_lr):
    x = np.ascontiguousarray(x, dtype=np.float32)
    rfl = np.ascontiguousarray(random_flow_lr, dtype=np.float32)
    cst = _build_consts()
    in_maps = []
    for k in range(NCORES):
        xsb = _stage_panels(x[k * S:(k + 1) * S]).reshape(XSN, 1)
        in_maps.append({"xs": xsb, "flow": rfl[k * S:(k + 1) * S], "cst": cst})
    return in_maps


def kernel(x, random_flow_lr):
    nc = get_nc()
    in_maps = make_in_maps(x, random_flow_lr)
    res = run_bass_kernel_spmd(nc, in_maps, core_ids=list(range(NCORES)))
    return np.concatenate([r["y"] for r in res.results], axis=0)
